# revision 2
# baseline (speedup 1.0000x reference)
"""GQA sparse-attention kernel for 8 Trainium2 NeuronCores — v3.

Sharding: data-parallel over batch (2) x sequence-parallel over query rows
(rows j::4 interleaved). No collectives.

v3 changes vs v2 (268us baseline):
  - perfect PSUM bank packing: the 16 causal s-blocks of each head pack as
    width-pairs summing to 512 (512 | 480+32 | ... | 288+224 | 256+256'),
    so a head PAIR fills exactly 17 one-bank [128,512] PSUM units with no
    padding.  One EXP and one mask-multiply per unit (8.5/head vs 10/head,
    all full-width).
  - deep software pipeline: psim pool of 5 units + shared pv/broadcast pool
    of 3 banks (8 total).  PV work is issued LAG=3 units behind sim, across
    head/pair boundaries, so PE semaphores are pre-satisfied: stalls reset
    the TRN2 PE DVFS ramp (measured: 512-col matmuls run 427ns at the 1.2GHz
    mid-pstate vs 216ns at 2.4GHz; ~4us of uninterrupted busy re-ramps).
  - tail per head: den-row copy (partition 64->64), PE broadcast matmul into
    the shared pv/bc pool, reciprocal_approx_fast, one DVE multiply.
  - host pre-tiles every weight/activation DRAM tensor into its SBUF layout
    so all input DMAs are contiguous >=4KB lines; ~13 big transfers round-
    robined over both HWDGE queues in consumption order (xk st-sliced so
    k-proj starts after ~1.5MB), wq/wv/consts on the SWDGE queue in parallel.
  - epilogue: stats chains + sqrt/recip LN chain, f16 LN apply, nt-outer
    out-projection so y stores stream to DRAM during compute.

Known behavior: run-to-run HW variance is large (~235-290us) — the PE DVFS
state interacts with cross-core power/HBM contention; medians ~250us.
"""

import os
import sys

import numpy as np

for _p in ("/opt/trn_rl_repo", "/root/.axon_site/_ro/trn_rl_repo"):
    if os.path.isdir(_p) and _p not in sys.path:
        sys.path.insert(0, _p)

B, N, E = 2, 2048, 1024
HQ, HK, D = 16, 4, 64
G = HQ // HK          # 4 query heads per kv head
KVE = HK * D          # 256
NL = N // 4           # 512 local query rows per core
SB = N // 128         # 16 s-blocks
EC = E // 128         # 8 embedding chunks
LN_EPS = 1e-5

# ---- head-pair PSUM packing ------------------------------------------------
# block k covers s in [128k, 128k+128); local query rows n >= 32k (global row
# 4n+j >= 128k), width w_k = 512-32k.  Emission orders pack banks exactly:
ORDER_EVEN = [0, 1, 15, 2, 14, 3, 13, 4, 12, 5, 11, 6, 10, 7, 9, 8]
ORDER_ODD = [8] + ORDER_EVEN[:-1]
NT = 17                                 # [128,512] PSUM units per head-pair
PAIR_COLS = NT * 512                    # 8704 layout cols, zero padding


def _build_pack():
    pack = {}
    pos = 0
    for par, order in ((0, ORDER_EVEN), (1, ORDER_ODD)):
        for k in order:
            w = 512 - 32 * k
            pack[(par, k)] = (pos // 512, pos % 512, 32 * k, w)
            pos += w
    assert pos == 8704
    return pack


PACK = _build_pack()
TW = [512] * NT
# unit -> [(par, k, off_in_unit, n0, w)] in layout order
TILE_BLOCKS = [[] for _ in range(NT)]
for (par, k), (t, off, n0, w) in sorted(PACK.items(), key=lambda i: (i[1][0], i[1][1])):
    TILE_BLOCKS[t].append((par, k, off, n0, w))
# PV emission per unit: entries are (par, k, src_unit, off, n0, w); odd
# head's k=8 PV is deferred to unit 9 so its k=0 start matmul (which zeroes
# the whole pv bank) always comes first
PV_BLOCKS = [[(par, k, t, off, n0, w) for par, k, off, n0, w in TILE_BLOCKS[t]]
             for t in range(NT)]
_odd8 = next(b for b in PV_BLOCKS[8] if b[0] == 1)
PV_BLOCKS[8].remove(_odd8)
PV_BLOCKS[9] = sorted(PV_BLOCKS[9] + [_odd8],
                      key=lambda b: (0 if b[1] == 0 else 1, b[3]))

_PROG_CACHE = {}


def build_program():
    import concourse.mybir as mybir
    import concourse.tile as tile
    from concourse import bacc

    dt = mybir.dt
    f32, f32r, f16, bf16 = dt.float32, dt.float32r, dt.float16, dt.bfloat16
    AF = mybir.ActivationFunctionType
    OP = mybir.AluOpType

    nc = bacc.Bacc("TRN2", target_bir_lowering=False, debug=False)

    def din(name, shape, dtp=f32):
        return nc.dram_tensor(name, shape, dtp, kind="ExternalInput").ap()

    # pre-tiled on host to the SBUF layout: [128, ...] with >=4KB lines
    xq_d = din("xq_d", [128, EC * NL], f16)
    xk_d = din("xk_d", [128, 4 * EC * 512], f16)   # st-major
    xv_d = din("xv_d", [128, 2 * EC * 1024], f16)  # half-major
    wq_d = din("wq_d", [128, EC * E], f16)         # pre-scaled by 1/8 on host
    wk_d = din("wk_d", [128, EC * KVE], f16)
    wv_d = din("wv_d", [128, EC * KVE], f16)
    wo_d = din("wo_d", [128, EC * E], f16)
    bq2d = din("bq2d", [EC, 128])          # bq/8
    bk2d = din("bk2d", [2, 128])
    bv2d = din("bv2d", [2, 128])
    bo1 = din("bo1", [1, E], f16)
    lng = din("lng", [EC, 128])
    lnb = din("lnb", [EC, 128])
    ones1 = din("ones1", [1, 128], f32r)
    ones1h = din("ones1h", [1, 128], f16)
    mask9 = din("mask9", [128, PAIR_COLS], f16)
    y = nc.dram_tensor("y", [NL, E], f32, kind="ExternalOutput").ap()

    # round-robin the two HWDGE queues in issue order
    def hw_q(i):
        return nc.sync if i % 2 == 0 else nc.scalar

    with tile.TileContext(nc) as tc, nc.allow_low_precision(
            "f16/f32r operands for PE fast-path matmuls are intentional"):
        with (
            tc.tile_pool(name="const", bufs=1) as pc,
            tc.tile_pool(name="persist", bufs=1) as pp,
        ):
            # ---- consolidated input DMAs, round-robin on the 2 HWDGE queues,
            # in consumption-priority order (each dma_start costs ~0.6us of
            # engine issue time, so few big transfers beat many small ones) ----
            qi = 0
            wk_all = pc.tile([128, EC * KVE], f16, tag="wk_all", name="wk_all")
            wk_sb = [wk_all[:, e * KVE:(e + 1) * KVE] for e in range(EC)]
            hw_q(qi).dma_start(wk_all[:], wk_d); qi += 1
            # xk in 4 st-slice tiles so k-proj st=0 starts after ~1.5MB
            xk_st = [pc.tile([128, EC * 512], f16, tag=f"xk_st{st}",
                             name=f"xk_st{st}") for st in range(4)]
            xk_sl = [[xk_st[st][:, e * 512:(e + 1) * 512]
                      for st in range(4)] for e in range(EC)]
            for st in range(4):
                hw_q(qi).dma_start(
                    xk_st[st][:], xk_d[:, st * EC * 512:(st + 1) * EC * 512])
                qi += 1
            wq_all = pc.tile([128, EC * E], f16, tag="wq_all", name="wq_all")
            wq_sb = [wq_all[:, e * E:(e + 1) * E] for e in range(EC)]
            nc.gpsimd.dma_start(wq_all[:], wq_d)
            xq_all = pc.tile([128, EC * NL], f16, tag="xq_all", name="xq_all")
            xq_sb = [xq_all[:, e * NL:(e + 1) * NL] for e in range(EC)]
            hw_q(qi).dma_start(xq_all[:], xq_d); qi += 1
            wv_all = pc.tile([128, EC * KVE], f16, tag="wv_all", name="wv_all")
            wv_sb = [wv_all[:, e * KVE:(e + 1) * KVE] for e in range(EC)]
            nc.gpsimd.dma_start(wv_all[:], wv_d)
            xv_st = [pc.tile([128, EC * 1024], f16, tag=f"xv_st{h}",
                             name=f"xv_st{h}") for h in range(2)]
            xv_sl = [[xv_st[st // 2][:, e * 1024 + (st % 2) * 512:
                                     e * 1024 + (st % 2) * 512 + 512]
                      for st in range(4)] for e in range(EC)]
            for half in range(2):
                hw_q(qi).dma_start(
                    xv_st[half][:],
                    xv_d[:, half * EC * 1024:(half + 1) * EC * 1024])
                qi += 1
            mask_sb = pp.tile([128, 8704], f16, tag="mask9", name="mask9")
            for half in range(2):
                hw_q(qi).dma_start(mask_sb[:, half * 4352:(half + 1) * 4352],
                                   mask9[:, half * 4352:(half + 1) * 4352])
                qi += 1
            wo_all = pp.tile([128, EC * E], f16, tag="wo_all", name="wo_all")
            wo_sb = [wo_all[:, e * E:(e + 1) * E] for e in range(EC)]
            hw_q(qi).dma_start(wo_all[:], wo_d); qi += 1

            # ---- tiny constants on the SWDGE queue ----
            bq_sb = [pc.tile([128, 1], f32, tag=f"bq{m}", name=f"bq{m}")
                     for m in range(EC)]
            for e in range(EC):
                nc.gpsimd.dma_start(bq_sb[e][:], bq2d[e:e + 1, :])
            bk_sb = [pc.tile([128, 1], f32, tag=f"bk{m}", name=f"bk{m}") for m in range(2)]
            bv_sb = [pc.tile([128, 1], f32, tag=f"bv{m}", name=f"bv{m}") for m in range(2)]
            for m in range(2):
                nc.gpsimd.dma_start(bk_sb[m][:], bk2d[m:m + 1, :])
                nc.gpsimd.dma_start(bv_sb[m][:], bv2d[m:m + 1, :])
            ones_k1 = pc.tile([1, 128], f32r, tag="ones_k1")
            nc.gpsimd.dma_start(ones_k1[:], ones1)
            ones64r = pc.tile([65, 64], f32r, tag="ones64r")
            nc.gpsimd.dma_start(ones64r[64:65, :], ones1[:, 0:64])
            ones_m1 = pc.tile([128, 1], f16, tag="ones_m1")
            nc.gpsimd.dma_start(ones_m1[:], ones1h)
            ones_k1h = pc.tile([1, 128], f16, tag="ones_k1h")
            nc.gpsimd.dma_start(ones_k1h[:], ones1h)
            lng_sb = [pp.tile([128, 1], f32, tag=f"lng{e}", name=f"lng{e}")
                      for e in range(EC)]
            lnb_sb = [pp.tile([128, 1], f32, tag=f"lnb{e}", name=f"lnb{e}")
                      for e in range(EC)]
            for e in range(EC):
                nc.gpsimd.dma_start(lng_sb[e][:], lng[e:e + 1, :])
                nc.gpsimd.dma_start(lnb_sb[e][:], lnb[e:e + 1, :])
            bo_sb = pp.tile([1, E], f16, tag="bo", name="bo")
            nc.gpsimd.dma_start(bo_sb[:], bo1)

            ident = pc.tile([128, 128], f16, tag="ident")
            from concourse.masks import make_identity
            make_identity(nc, ident[:])
            eps_c = pc.tile([1, 1], f32, tag="eps_c")
            nc.gpsimd.memset(eps_c[:], LN_EPS)

            # persistent activation tiles
            kT_sb = [pp.tile([128, N], f16, tag=f"kt{m}", name=f"kt{m}") for m in range(2)]
            v_ext = [pp.tile([128, 4 * 128], f16, tag=f"vx{k}", name=f"vx{k}")
                     for k in range(SB)]
            qp_sb = [pp.tile([128, NL], f16, tag=f"qp{m}", name=f"qp{m}")
                     for m in range(EC)]
            attT = [pp.tile([128, NL], f16, tag=f"at{e}", name=f"at{e}")
                    for e in range(EC)]
            _EVEN = [0, 1, 2, 3, 8, 9, 10, 11]    # heads whose kv head is even
            _ODD = [4, 5, 6, 7, 12, 13, 14, 15]

            def _qslot(g):
                if (g // G) % 2 == 0:
                    return _EVEN.index(g), 0
                return _ODD.index(g), 1

            # ---------------- phase 0: k projection ----------------
            with tc.tile_pool(name="psk2", bufs=2, space="PSUM") as psk2:
                for st in range(4):
                    sl = slice(st * 512, (st + 1) * 512)
                    for mt in range(2):
                        psk = psk2.tile([128, 512], f32, tag="psk")
                        for e in range(EC):
                            nc.tensor.matmul(
                                psk[:], wk_sb[e][:, mt * 128:(mt + 1) * 128],
                                xk_sl[e][st], start=(e == 0), stop=(e == EC - 1))
                        nc.scalar.activation(kT_sb[mt][:, sl], psk[:], AF.Identity,
                                             bias=bk_sb[mt][:], scale=1.0)

            # ---------------- phase 1: q projection ----------------
            with tc.tile_pool(name="psq", bufs=2, space="PSUM") as psq:
                for mt2 in range(EC // 2):
                    psA = psq.tile([128, NL], f32, tag="psqA", name="psqA")
                    psB = psq.tile([128, NL], f32, tag="psqB", name="psqB")
                    for e in range(EC):
                        for mt, ps in ((2 * mt2, psA), (2 * mt2 + 1, psB)):
                            nc.tensor.matmul(
                                ps[:], wq_sb[e][:, mt * 128:(mt + 1) * 128],
                                xq_sb[e], start=(e == 0), stop=(e == EC - 1))
                    for mt, ps in ((2 * mt2, psA), (2 * mt2 + 1, psB)):
                        for t in range(2):
                            g = 2 * mt + t
                            ti, slot = _qslot(g)
                            nc.scalar.activation(
                                qp_sb[ti][slot * 64:(slot + 1) * 64, :],
                                ps[t * 64:(t + 1) * 64, :], AF.Identity,
                                bias=bq_sb[mt][t * 64:(t + 1) * 64, :], scale=1.0)

            # ---------------- phase 2: v projection ----------------
            with (
                tc.tile_pool(name="vt", bufs=2) as pvt,
                tc.tile_pool(name="pskv", bufs=2, space="PSUM") as pskv,
                tc.tile_pool(name="pst", bufs=2, space="PSUM") as pst,
            ):
                for st in range(4):  # s-tiles of 512
                    for mt in range(2):
                        psv = pskv.tile([128, 512], f32, tag="psv")
                        for e in range(EC):
                            nc.tensor.matmul(
                                psv[:], wv_sb[e][:, mt * 128:(mt + 1) * 128],
                                xv_sl[e][st], start=(e == 0), stop=(e == EC - 1))
                        vt = pvt.tile([128, 512], f16, tag="vt")
                        nc.scalar.activation(vt[:], psv[:], AF.Identity,
                                             bias=bv_sb[mt][:], scale=1.0)
                        for ss in range(4):
                            k = st * 4 + ss
                            pt = pst.tile([128, 128], f16, tag="pt")
                            nc.tensor.transpose(pt[:], vt[:, ss * 128:(ss + 1) * 128],
                                                ident[:])
                            src = pt[:].rearrange("p (h x) -> p h x", h=2)
                            dst = v_ext[k][:].rearrange("p (h x) -> p h x", h=4)
                            nc.vector.tensor_copy(dst[:, 2 * mt:2 * mt + 2, 0:64], src)
                for k in range(SB):
                    pad = v_ext[k][:].rearrange("p (h x) -> p h x", h=4)[:, :, 64:128]
                    nc.gpsimd.memset(pad, 0.0)
                    one_col = v_ext[k][:].rearrange("p (h x) -> p h x", h=4)[:, :, 64:65]
                    nc.gpsimd.memset(one_col, 1.0)

            # ---------------- phase 3: attention ----------------
            # 512-col PSUM units, PV issued LAG units behind sim so every PE
            # semaphore is pre-satisfied and the DVFS busy-streak stays long
            LAG = 3
            with (
                tc.tile_pool(name="exs", bufs=LAG + 3) as pex,
                tc.tile_pool(name="recs", bufs=2) as prec,
                tc.tile_pool(name="psim", bufs=5, space="PSUM") as psim,
                tc.tile_pool(name="pvbc", bufs=3, space="PSUM") as pvbc,
            ):
                def make_pair(m):
                    """Returns per-unit work closures for head pair (2m, 2m+1)."""
                    gs = (2 * m, 2 * m + 1)
                    hs = tuple(g // G for g in gs)
                    kh = []
                    qg = []
                    for g in gs:
                        h = g // G
                        ti, slot = _qslot(g)
                        qg.append(qp_sb[ti][slot * 64:(slot + 1) * 64, :])
                        kh.append(kT_sb[h // 2][(h % 2) * 64:(h % 2) * 64 + 64, :])
                    pvs = [None, None]
                    sims = [None] * NT
                    exs = [None] * NT

                    def do_sim(t):
                        st_ = psim.tile([128, 512], f32, tag="sim", name="sim")
                        sims[t] = st_
                        for par, k, off, n0, w in TILE_BLOCKS[t]:
                            nc.tensor.matmul(
                                st_[:, off:off + w],
                                kh[par][:, k * 128:(k + 1) * 128],
                                qg[par][:, n0:512], start=True, stop=True)

                    def do_exp(t):
                        ex = pex.tile([128, 512], f16, tag="ex", name="ex")
                        exs[t] = ex
                        nc.scalar.activation(ex[:], sims[t][:], AF.Exp)
                        eng = nc.gpsimd if t % 4 == 2 else nc.vector
                        eng.tensor_tensor(
                            ex[:], ex[:],
                            mask_sb[:, t * 512:(t + 1) * 512], op=OP.mult)

                    def do_pv(t):
                        for par, k, src, off, n0, w in PV_BLOCKS[t]:
                            h = hs[par]
                            if k == 0:
                                pvs[par] = pvbc.tile([128, 512], f32, tag="pv",
                                                     name="pv")
                            nc.tensor.matmul(
                                pvs[par][:, n0:512],
                                v_ext[k][:, 128 * h:128 * h + 128],
                                exs[src][:, off:off + w],
                                start=(k == 0), stop=(k == (8 if par == 0 else 9)),
                                skip_group_check=True)

                    def tail(par):
                        # v2-proven chain: den row copy (64->64), PE broadcast
                        # to partitions 0-63, fast reciprocal, one DVE mult
                        g = gs[par]
                        pv = pvs[par]
                        den = prec.tile([65, 512], f32r, tag="den", name="den")
                        nc.vector.tensor_copy(den[64:65, :], pv[64:65, :])
                        bct = pvbc.tile([128, 512], f32, tag="pv", name="bct")
                        bc = bct[0:64, :]
                        nc.tensor.matmul(bc, ones64r[64:65, :], den[64:65, :],
                                         start=True, stop=True)
                        rec = prec.tile([64, 512], f32, tag="rec", name="rec")
                        nc.vector.reciprocal_approx_fast(rec[:], bc)
                        p0 = (g % 2) * 64
                        att = attT[g // 2][p0:p0 + 64, :]
                        nc.vector.tensor_tensor(att, pv[0:64, :], rec[:],
                                                op=OP.mult)

                    return do_sim, do_exp, do_pv, tail

                stream = []
                for m in range(8):
                    do_sim, do_exp, do_pv, tail = make_pair(m)
                    for t in range(NT):
                        post = []
                        if t == 8:
                            post.append(lambda tl=tail: tl(0))
                        if t == 16:
                            post.append(lambda tl=tail: tl(1))
                        stream.append((do_sim, do_exp, do_pv, t, post))
                for i, unit in enumerate(stream):
                    do_sim, do_exp, do_pv, t, post = unit
                    do_sim(t)
                    do_exp(t)
                    if i >= LAG:
                        p_sim, p_exp, p_pv, p_t, p_post = stream[i - LAG]
                        p_pv(p_t)
                        for fn in p_post:
                            fn()
                for i in range(len(stream) - LAG, len(stream)):
                    p_sim, p_exp, p_pv, p_t, p_post = stream[i]
                    p_pv(p_t)
                    for fn in p_post:
                        fn()

            # ---------------- phase 4: layernorm + out projection ----------------
            with (tc.tile_pool(name="lnt", bufs=1) as plnt,
                  tc.tile_pool(name="lnx", bufs=2) as plnx):
                mb16 = plnt.tile([128, NL], f16, tag="mb16")
                ib16 = plnt.tile([128, NL], f16, tag="ib16")
                with tc.tile_pool(name="psst", bufs=1, space="PSUM") as psst:
                    st_sum = psst.tile([1, NL], f32, tag="ssum")
                    st_sq = psst.tile([1, NL], f32, tag="ssq")
                    for e in range(EC):
                        sq = plnx.tile([128, NL], f16, tag="sqe")
                        nc.vector.tensor_tensor(sq[:], attT[e][:], attT[e][:],
                                                op=OP.mult)
                        nc.tensor.matmul(st_sum[:], ones_m1[:], attT[e][:],
                                         start=(e == 0), stop=(e == EC - 1))
                        nc.tensor.matmul(st_sq[:], ones_m1[:], sq[:],
                                         start=(e == 0), stop=(e == EC - 1))
                    mu = plnt.tile([1, NL], f32r, tag="mu")
                    nc.vector.tensor_scalar_mul(mu[:], st_sum[:], 1.0 / E)
                    var = plnt.tile([1, NL], f32, tag="var")
                    nc.vector.tensor_scalar_mul(var[:], st_sq[:], 1.0 / E)
                    mu2 = plnt.tile([1, NL], f32, tag="mu2")
                    nc.vector.tensor_tensor(mu2[:], mu[:], mu[:], op=OP.mult)
                    nc.vector.tensor_tensor(var[:], var[:], mu2[:], op=OP.subtract)
                    sd = plnt.tile([1, NL], f32, tag="sd")
                    nc.scalar.activation(sd[:], var[:], AF.Sqrt, bias=eps_c[:])
                    nc.vector.reciprocal_approx_fast(sd[:], sd[:])
                    isd = plnt.tile([1, NL], f32r, tag="isd")
                    nc.vector.tensor_copy(isd[:], sd[:])
                    mb = psst.tile([128, NL], f32, tag="mb")
                    nc.tensor.matmul(mb[:], ones_k1[:], mu[:], start=True, stop=True)
                    ib = psst.tile([128, NL], f32, tag="ib")
                    nc.tensor.matmul(ib[:], ones_k1[:], isd[:], start=True, stop=True)
                    nc.vector.tensor_copy(mb16[:], mb[:])
                    nc.vector.tensor_copy(ib16[:], ib[:])

                def apply_ln(e):
                    tmp = plnx.tile([128, NL], f16, tag="xn")
                    nc.vector.tensor_tensor(tmp[:], attT[e][:], mb16[:],
                                            op=OP.subtract)
                    nc.vector.tensor_tensor(tmp[:], tmp[:], ib16[:], op=OP.mult)
                    nc.vector.tensor_scalar(attT[e][:], tmp[:], lng_sb[e][:],
                                            lnb_sb[e][:], op0=OP.mult, op1=OP.add)

                for e in range(EC):
                    apply_ln(e)
                with (
                    tc.tile_pool(name="ysb", bufs=2) as pysb,
                    tc.tile_pool(name="psy", bufs=2, space="PSUM") as psy,
                ):
                    qi2 = 0
                    for nt in range(4):
                        pyA = psy.tile([128, 512], f32, tag="pyA", name="pyA")
                        pyB = psy.tile([128, 512], f32, tag="pyB", name="pyB")
                        pys2 = (pyA, pyB)
                        for e in range(EC):
                            for oc in range(2):
                                nc.tensor.matmul(
                                    pys2[oc][:],
                                    attT[e][:, nt * 128:(nt + 1) * 128],
                                    wo_sb[e][:, oc * 512:(oc + 1) * 512],
                                    start=(e == 0), stop=False)
                        for oc in range(2):
                            nc.tensor.matmul(pys2[oc][:], ones_k1h[:],
                                             bo_sb[0:1, oc * 512:(oc + 1) * 512],
                                             start=False, stop=True)
                            ys = pysb.tile([128, 512], f32, tag="ys")
                            if oc == 0:
                                nc.vector.tensor_copy(ys[:], pys2[oc][:])
                            else:
                                nc.scalar.activation(ys[:], pys2[oc][:],
                                                     AF.Identity)
                            hw_q(qi2).dma_start(
                                y[nt * 128:(nt + 1) * 128,
                                  oc * 512:(oc + 1) * 512], ys[:])
                            qi2 += 1
    nc.finalize()
    return nc


def _build_mask9(adjc):
    """adjc: [NL, N] int (causal&adj premasked). Returns the [128, PAIR_COLS]
    f16 mask in the pair layout."""
    at = adjc.T.astype(np.float16)  # [N, NL] = [s, n]
    mk = np.zeros((128, PAIR_COLS), np.float16)
    for (par, k), (t, off, n0, w) in PACK.items():
        mk[:, t * 512 + off:t * 512 + off + w] = at[k * 128:(k + 1) * 128, n0:512]
    return mk


def _tile_w(wT):
    """[E, M] f16 -> [128, EC*M] with [p, e*M+m] = wT[e*128+p, m]."""
    E_, M = wT.shape
    return np.ascontiguousarray(
        wT.reshape(EC, 128, M).transpose(1, 0, 2).reshape(128, EC * M))


def shard_inputs(inputs):
    q = np.asarray(inputs["query"], np.float32)
    k = np.asarray(inputs["key"], np.float32)
    v = np.asarray(inputs["value"], np.float32)
    adj = np.asarray(inputs["adj"], np.int32)
    WqT8 = (np.ascontiguousarray(np.asarray(inputs["Wq"], np.float32).T)
            / np.float32(8.0)).astype(np.float16)
    WkT = np.ascontiguousarray(np.asarray(inputs["Wk"], np.float32).T).astype(np.float16)
    WvT = np.ascontiguousarray(np.asarray(inputs["Wv"], np.float32).T).astype(np.float16)
    WoT = np.ascontiguousarray(np.asarray(inputs["Wo"], np.float32).T).astype(np.float16)
    bq8 = (np.asarray(inputs["bq"], np.float32) / np.float32(8.0)).reshape(EC, 128)
    bk2 = np.asarray(inputs["bk"], np.float32).reshape(2, 128)
    bv2 = np.asarray(inputs["bv"], np.float32).reshape(2, 128)
    bo1 = np.asarray(inputs["bo"], np.float32).reshape(1, E).astype(np.float16)
    lng = np.asarray(inputs["ln_g"], np.float32).reshape(EC, 128)
    lnb = np.asarray(inputs["ln_b"], np.float32).reshape(EC, 128)

    shared = dict(wq_d=_tile_w(WqT8), wk_d=_tile_w(WkT), wv_d=_tile_w(WvT),
                  wo_d=_tile_w(WoT), bq2d=bq8, bk2d=bk2,
                  bv2d=bv2, bo1=bo1, lng=lng, lnb=lnb,
                  ones1=np.ones((1, 128), np.float32),
                  ones1h=np.ones((1, 128), np.float16))
    per_b = []
    s_idx = np.arange(N)
    for b in range(B):
        kT = k[b].T.astype(np.float16).reshape(EC, 128, 4, 512)
        xk_d = np.ascontiguousarray(
            kT.transpose(1, 2, 0, 3).reshape(128, 4 * EC * 512))
        vT = v[b].T.astype(np.float16).reshape(EC, 128, 2, 1024)
        xv_d = np.ascontiguousarray(
            vT.transpose(1, 2, 0, 3).reshape(128, 2 * EC * 1024))
        per_b.append((xk_d, xv_d))
    in_maps = []
    for c in range(8):
        b, j = divmod(c, 4)
        rows = np.arange(j, N, 4)
        causal = s_idx[None, :] <= rows[:, None]          # [NL, N]
        adjc = np.where(causal, adj[b][rows], 0)
        m = dict(shared)
        m["xq_d"] = _tile_w(q[b][rows].T.astype(np.float16))
        m["xk_d"], m["xv_d"] = per_b[b]
        m["mask9"] = _build_mask9(adjc)
        in_maps.append(m)
    return in_maps


def _host_fixup(out, inputs):
    """Rows with no unmasked causal position get the reference's uniform-
    softmax-over-everything fallback, computed exactly on host."""
    adj = np.asarray(inputs["adj"])
    s_idx = np.arange(N)
    causal = s_idx[None, :] <= s_idx[:, None]
    for b in range(B):
        amr = np.where(((adj[b] != 0) & causal).sum(1) == 0)[0]
        if len(amr) == 0:
            continue
        v = np.asarray(inputs["value"][b], np.float64)
        Wv = np.asarray(inputs["Wv"], np.float64)
        bv = np.asarray(inputs["bv"], np.float64)
        vp = v @ Wv.T + bv                       # [N, KVE]
        mv = vp.mean(0)                          # [KVE]
        row = np.concatenate([mv[(k // G) * D:(k // G) * D + D] for k in range(HQ)])
        mu = row.mean()
        var = ((row - mu) ** 2).mean()
        rown = (row - mu) / np.sqrt(var + LN_EPS)
        rown = rown * np.asarray(inputs["ln_g"], np.float64) + np.asarray(
            inputs["ln_b"], np.float64)
        yrow = rown @ np.asarray(inputs["Wo"], np.float64).T + np.asarray(
            inputs["bo"], np.float64)
        out[b, amr, :] = yrow.astype(np.float32)
    return out


def unshard_outputs(results, inputs):
    out = np.empty((B, N, E), np.float32)
    for c in range(8):
        b, j = divmod(c, 4)
        out[b, j::4, :] = results[c]["y"]
    return _host_fixup(out, inputs)


def kernel(**inputs):
    from concourse.bass_utils import run_bass_kernel_spmd

    if "nc" not in _PROG_CACHE:
        _PROG_CACHE["nc"] = build_program()
    nc = _PROG_CACHE["nc"]
    in_maps = shard_inputs(inputs)
    res = run_bass_kernel_spmd(nc, in_maps, core_ids=list(range(8)))
    return unshard_outputs(res.results, inputs)


# revision 3
# speedup vs baseline: 1.0053x; 1.0053x over previous
"""GQA sparse-attention kernel for 8 Trainium2 NeuronCores — v3.

Sharding: data-parallel over batch (2) x sequence-parallel over query rows
(rows j::4 interleaved). No collectives.

v3 changes vs v2 (268us baseline):
  - perfect PSUM bank packing: the 16 causal s-blocks of each head pack as
    width-pairs summing to 512 (512 | 480+32 | ... | 288+224 | 256+256'),
    so a head PAIR fills exactly 17 one-bank [128,512] PSUM units with no
    padding.  One EXP and one mask-multiply per unit (8.5/head vs 10/head,
    all full-width).
  - deep software pipeline: psim pool of 5 units + shared pv/broadcast pool
    of 3 banks (8 total).  PV work is issued LAG=3 units behind sim, across
    head/pair boundaries, so PE semaphores are pre-satisfied: stalls reset
    the TRN2 PE DVFS ramp (measured: 512-col matmuls run 427ns at the 1.2GHz
    mid-pstate vs 216ns at 2.4GHz; ~4us of uninterrupted busy re-ramps).
  - tail per head: den-row copy (partition 64->64), PE broadcast matmul into
    the shared pv/bc pool, reciprocal_approx_fast, one DVE multiply.
  - host pre-tiles every weight/activation DRAM tensor into its SBUF layout
    so all input DMAs are contiguous >=4KB lines; ~13 big transfers round-
    robined over both HWDGE queues in consumption order (xk st-sliced so
    k-proj starts after ~1.5MB), wq/wv/consts on the SWDGE queue in parallel.
  - epilogue: stats chains + sqrt/recip LN chain, f16 LN apply, nt-outer
    out-projection so y stores stream to DRAM during compute.

Known behavior: run-to-run HW variance is large (~235-290us) — the PE DVFS
state interacts with cross-core power/HBM contention; medians ~250us.
"""

import os
import sys

import numpy as np

for _p in ("/opt/trn_rl_repo", "/root/.axon_site/_ro/trn_rl_repo"):
    if os.path.isdir(_p) and _p not in sys.path:
        sys.path.insert(0, _p)

B, N, E = 2, 2048, 1024
HQ, HK, D = 16, 4, 64
G = HQ // HK          # 4 query heads per kv head
KVE = HK * D          # 256
NL = N // 4           # 512 local query rows per core
SB = N // 128         # 16 s-blocks
EC = E // 128         # 8 embedding chunks
LN_EPS = 1e-5

# ---- head-pair PSUM packing ------------------------------------------------
# block k covers s in [128k, 128k+128); local query rows n >= 32k (global row
# 4n+j >= 128k), width w_k = 512-32k.  Emission orders pack banks exactly:
ORDER_EVEN = [0, 1, 15, 2, 14, 3, 13, 4, 12, 5, 11, 6, 10, 7, 9, 8]
ORDER_ODD = [8] + ORDER_EVEN[:-1]
NT = 17                                 # [128,512] PSUM units per head-pair
PAIR_COLS = NT * 512                    # 8704 layout cols, zero padding


def _build_pack():
    pack = {}
    pos = 0
    for par, order in ((0, ORDER_EVEN), (1, ORDER_ODD)):
        for k in order:
            w = 512 - 32 * k
            pack[(par, k)] = (pos // 512, pos % 512, 32 * k, w)
            pos += w
    assert pos == 8704
    return pack


PACK = _build_pack()
TW = [512] * NT
# unit -> [(par, k, off_in_unit, n0, w)] in layout order
TILE_BLOCKS = [[] for _ in range(NT)]
for (par, k), (t, off, n0, w) in sorted(PACK.items(), key=lambda i: (i[1][0], i[1][1])):
    TILE_BLOCKS[t].append((par, k, off, n0, w))
# PV emission per unit: entries are (par, k, src_unit, off, n0, w); odd
# head's k=8 PV is deferred to unit 9 so its k=0 start matmul (which zeroes
# the whole pv bank) always comes first
PV_BLOCKS = [[(par, k, t, off, n0, w) for par, k, off, n0, w in TILE_BLOCKS[t]]
             for t in range(NT)]
_odd8 = next(b for b in PV_BLOCKS[8] if b[0] == 1)
PV_BLOCKS[8].remove(_odd8)
PV_BLOCKS[9] = sorted(PV_BLOCKS[9] + [_odd8],
                      key=lambda b: (0 if b[1] == 0 else 1, b[3]))

_PROG_CACHE = {}


def build_program():
    import concourse.mybir as mybir
    import concourse.tile as tile
    from concourse import bacc

    dt = mybir.dt
    f32, f32r, f16, bf16 = dt.float32, dt.float32r, dt.float16, dt.bfloat16
    AF = mybir.ActivationFunctionType
    OP = mybir.AluOpType

    nc = bacc.Bacc("TRN2", target_bir_lowering=False, debug=False)

    def din(name, shape, dtp=f32):
        return nc.dram_tensor(name, shape, dtp, kind="ExternalInput").ap()

    # pre-tiled on host to the SBUF layout: [128, ...] with >=4KB lines
    xq_d = din("xq_d", [128, EC * NL], f16)
    xk_d = din("xk_d", [128, 4 * EC * 512], f16)   # st-major
    xv_d = din("xv_d", [128, 2 * EC * 1024], f16)  # half-major
    wq_d = din("wq_d", [128, EC * E], f16)         # pre-scaled by 1/8 on host
    wk_d = din("wk_d", [128, EC * KVE], f16)
    wv_d = din("wv_d", [128, EC * KVE], f16)
    wo_d = din("wo_d", [128, EC * E], f16)
    bq2d = din("bq2d", [EC, 128])          # bq/8
    bk2d = din("bk2d", [2, 128])
    bv2d = din("bv2d", [2, 128])
    bo1 = din("bo1", [1, E], f16)
    lng = din("lng", [EC, 128])
    lnb = din("lnb", [EC, 128])
    ones1 = din("ones1", [1, 128], f32r)
    ones1h = din("ones1h", [1, 128], f16)
    mask9 = din("mask9", [128, PAIR_COLS], f16)
    y = nc.dram_tensor("y", [NL, E], f32, kind="ExternalOutput").ap()

    # round-robin the two HWDGE queues in issue order
    def hw_q(i):
        return nc.sync if i % 2 == 0 else nc.scalar

    with tile.TileContext(nc) as tc, nc.allow_low_precision(
            "f16/f32r operands for PE fast-path matmuls are intentional"):
        with (
            tc.tile_pool(name="const", bufs=1) as pc,
            tc.tile_pool(name="persist", bufs=1) as pp,
        ):
            # ---- consolidated input DMAs, round-robin on the 2 HWDGE queues,
            # in consumption-priority order (each dma_start costs ~0.6us of
            # engine issue time, so few big transfers beat many small ones) ----
            qi = 0
            wk_all = pc.tile([128, EC * KVE], f16, tag="wk_all", name="wk_all")
            wk_sb = [wk_all[:, e * KVE:(e + 1) * KVE] for e in range(EC)]
            hw_q(qi).dma_start(wk_all[:], wk_d); qi += 1
            # xk in 4 st-slice tiles so k-proj st=0 starts after ~1.5MB
            xk_st = [pc.tile([128, EC * 512], f16, tag=f"xk_st{st}",
                             name=f"xk_st{st}") for st in range(4)]
            xk_sl = [[xk_st[st][:, e * 512:(e + 1) * 512]
                      for st in range(4)] for e in range(EC)]
            for st in range(4):
                hw_q(qi).dma_start(
                    xk_st[st][:], xk_d[:, st * EC * 512:(st + 1) * EC * 512])
                qi += 1
            wq_all = pc.tile([128, EC * E], f16, tag="wq_all", name="wq_all")
            wq_sb = [wq_all[:, e * E:(e + 1) * E] for e in range(EC)]
            nc.gpsimd.dma_start(wq_all[:], wq_d)
            xq_all = pc.tile([128, EC * NL], f16, tag="xq_all", name="xq_all")
            xq_sb = [xq_all[:, e * NL:(e + 1) * NL] for e in range(EC)]
            hw_q(qi).dma_start(xq_all[:], xq_d); qi += 1
            wv_all = pc.tile([128, EC * KVE], f16, tag="wv_all", name="wv_all")
            wv_sb = [wv_all[:, e * KVE:(e + 1) * KVE] for e in range(EC)]
            nc.gpsimd.dma_start(wv_all[:], wv_d)
            xv_st = [pc.tile([128, EC * 1024], f16, tag=f"xv_st{h}",
                             name=f"xv_st{h}") for h in range(2)]
            xv_sl = [[xv_st[st // 2][:, e * 1024 + (st % 2) * 512:
                                     e * 1024 + (st % 2) * 512 + 512]
                      for st in range(4)] for e in range(EC)]
            for half in range(2):
                hw_q(qi).dma_start(
                    xv_st[half][:],
                    xv_d[:, half * EC * 1024:(half + 1) * EC * 1024])
                qi += 1
            mask_sb = pp.tile([128, 8704], f16, tag="mask9", name="mask9")
            for half in range(2):
                hw_q(qi).dma_start(mask_sb[:, half * 4352:(half + 1) * 4352],
                                   mask9[:, half * 4352:(half + 1) * 4352])
                qi += 1
            wo_all = pp.tile([128, EC * E], f16, tag="wo_all", name="wo_all")
            wo_sb = [wo_all[:, e * E:(e + 1) * E] for e in range(EC)]
            hw_q(qi).dma_start(wo_all[:], wo_d); qi += 1

            # ---- tiny constants on the SWDGE queue ----
            bq_sb = [pc.tile([128, 1], f32, tag=f"bq{m}", name=f"bq{m}")
                     for m in range(EC)]
            for e in range(EC):
                nc.gpsimd.dma_start(bq_sb[e][:], bq2d[e:e + 1, :])
            bk_sb = [pc.tile([128, 1], f32, tag=f"bk{m}", name=f"bk{m}") for m in range(2)]
            bv_sb = [pc.tile([128, 1], f32, tag=f"bv{m}", name=f"bv{m}") for m in range(2)]
            for m in range(2):
                nc.gpsimd.dma_start(bk_sb[m][:], bk2d[m:m + 1, :])
                nc.gpsimd.dma_start(bv_sb[m][:], bv2d[m:m + 1, :])
            ones_k1 = pc.tile([1, 128], f32r, tag="ones_k1")
            nc.gpsimd.dma_start(ones_k1[:], ones1)
            ones64r = pc.tile([65, 64], f32r, tag="ones64r")
            nc.gpsimd.dma_start(ones64r[64:65, :], ones1[:, 0:64])
            ones_m1 = pc.tile([128, 1], f16, tag="ones_m1")
            nc.gpsimd.dma_start(ones_m1[:], ones1h)
            ones_k1h = pc.tile([1, 128], f16, tag="ones_k1h")
            nc.gpsimd.dma_start(ones_k1h[:], ones1h)
            lng_sb = [pp.tile([128, 1], f32, tag=f"lng{e}", name=f"lng{e}")
                      for e in range(EC)]
            lnb_sb = [pp.tile([128, 1], f32, tag=f"lnb{e}", name=f"lnb{e}")
                      for e in range(EC)]
            for e in range(EC):
                nc.gpsimd.dma_start(lng_sb[e][:], lng[e:e + 1, :])
                nc.gpsimd.dma_start(lnb_sb[e][:], lnb[e:e + 1, :])
            bo_sb = pp.tile([1, E], f16, tag="bo", name="bo")
            nc.gpsimd.dma_start(bo_sb[:], bo1)

            ident = pc.tile([128, 128], f16, tag="ident")
            from concourse.masks import make_identity
            make_identity(nc, ident[:])
            eps_c = pc.tile([1, 1], f32, tag="eps_c")
            nc.gpsimd.memset(eps_c[:], LN_EPS)

            # persistent activation tiles
            kT_sb = [pp.tile([128, N], f16, tag=f"kt{m}", name=f"kt{m}") for m in range(2)]
            v_ext = [pp.tile([128, 4 * 128], f16, tag=f"vx{k}", name=f"vx{k}")
                     for k in range(SB)]
            qp_sb = [pp.tile([128, NL], f16, tag=f"qp{m}", name=f"qp{m}")
                     for m in range(EC)]
            attT = [pp.tile([128, NL], f16, tag=f"at{e}", name=f"at{e}")
                    for e in range(EC)]
            _EVEN = [0, 1, 2, 3, 8, 9, 10, 11]    # heads whose kv head is even
            _ODD = [4, 5, 6, 7, 12, 13, 14, 15]

            def _qslot(g):
                if (g // G) % 2 == 0:
                    return _EVEN.index(g), 0
                return _ODD.index(g), 1

            # ---------------- phase 0: k projection ----------------
            with tc.tile_pool(name="psk2", bufs=2, space="PSUM") as psk2:
                for st in range(4):
                    sl = slice(st * 512, (st + 1) * 512)
                    for mt in range(2):
                        psk = psk2.tile([128, 512], f32, tag="psk")
                        for e in range(EC):
                            nc.tensor.matmul(
                                psk[:], wk_sb[e][:, mt * 128:(mt + 1) * 128],
                                xk_sl[e][st], start=(e == 0), stop=(e == EC - 1))
                        nc.scalar.activation(kT_sb[mt][:, sl], psk[:], AF.Identity,
                                             bias=bk_sb[mt][:], scale=1.0)

            # ---------------- phase 1: q projection ----------------
            with tc.tile_pool(name="psq", bufs=2, space="PSUM") as psq:
                for mt2 in range(EC // 2):
                    psA = psq.tile([128, NL], f32, tag="psqA", name="psqA")
                    psB = psq.tile([128, NL], f32, tag="psqB", name="psqB")
                    for e in range(EC):
                        for mt, ps in ((2 * mt2, psA), (2 * mt2 + 1, psB)):
                            nc.tensor.matmul(
                                ps[:], wq_sb[e][:, mt * 128:(mt + 1) * 128],
                                xq_sb[e], start=(e == 0), stop=(e == EC - 1))
                    for mt, ps in ((2 * mt2, psA), (2 * mt2 + 1, psB)):
                        for t in range(2):
                            g = 2 * mt + t
                            ti, slot = _qslot(g)
                            nc.scalar.activation(
                                qp_sb[ti][slot * 64:(slot + 1) * 64, :],
                                ps[t * 64:(t + 1) * 64, :], AF.Identity,
                                bias=bq_sb[mt][t * 64:(t + 1) * 64, :], scale=1.0)

            # ---------------- phase 2: v projection ----------------
            with (
                tc.tile_pool(name="vt", bufs=2) as pvt,
                tc.tile_pool(name="pskv", bufs=2, space="PSUM") as pskv,
                tc.tile_pool(name="pst", bufs=2, space="PSUM") as pst,
            ):
                for st in range(4):  # s-tiles of 512
                    for mt in range(2):
                        psv = pskv.tile([128, 512], f32, tag="psv")
                        for e in range(EC):
                            nc.tensor.matmul(
                                psv[:], wv_sb[e][:, mt * 128:(mt + 1) * 128],
                                xv_sl[e][st], start=(e == 0), stop=(e == EC - 1))
                        vt = pvt.tile([128, 512], f16, tag="vt")
                        nc.scalar.activation(vt[:], psv[:], AF.Identity,
                                             bias=bv_sb[mt][:], scale=1.0)
                        for ss in range(4):
                            k = st * 4 + ss
                            pt = pst.tile([128, 128], f16, tag="pt")
                            nc.tensor.transpose(pt[:], vt[:, ss * 128:(ss + 1) * 128],
                                                ident[:])
                            src = pt[:].rearrange("p (h x) -> p h x", h=2)
                            dst = v_ext[k][:].rearrange("p (h x) -> p h x", h=4)
                            nc.vector.tensor_copy(dst[:, 2 * mt:2 * mt + 2, 0:64], src)
                for k in range(SB):
                    pad = v_ext[k][:].rearrange("p (h x) -> p h x", h=4)[:, :, 64:128]
                    nc.gpsimd.memset(pad, 0.0)
                    one_col = v_ext[k][:].rearrange("p (h x) -> p h x", h=4)[:, :, 64:65]
                    nc.gpsimd.memset(one_col, 1.0)

            # ---------------- phase 3: attention ----------------
            # 512-col PSUM units, PV issued LAG units behind sim so every PE
            # semaphore is pre-satisfied and the DVFS busy-streak stays long
            LAG = 4
            with (
                tc.tile_pool(name="exs", bufs=LAG + 3) as pex,
                tc.tile_pool(name="recs", bufs=2) as prec,
                tc.tile_pool(name="psim", bufs=5, space="PSUM") as psim,
                tc.tile_pool(name="pvbc", bufs=3, space="PSUM") as pvbc,
            ):
                def make_pair(m):
                    """Returns per-unit work closures for head pair (2m, 2m+1)."""
                    gs = (2 * m, 2 * m + 1)
                    hs = tuple(g // G for g in gs)
                    kh = []
                    qg = []
                    for g in gs:
                        h = g // G
                        ti, slot = _qslot(g)
                        qg.append(qp_sb[ti][slot * 64:(slot + 1) * 64, :])
                        kh.append(kT_sb[h // 2][(h % 2) * 64:(h % 2) * 64 + 64, :])
                    pvs = [None, None]
                    sims = [None] * NT
                    exs = [None] * NT

                    def do_sim(t):
                        st_ = psim.tile([128, 512], f32, tag="sim", name="sim")
                        sims[t] = st_
                        for par, k, off, n0, w in TILE_BLOCKS[t]:
                            nc.tensor.matmul(
                                st_[:, off:off + w],
                                kh[par][:, k * 128:(k + 1) * 128],
                                qg[par][:, n0:512], start=True, stop=True)

                    def do_exp(t):
                        ex = pex.tile([128, 512], f16, tag="ex", name="ex")
                        exs[t] = ex
                        nc.scalar.activation(ex[:], sims[t][:], AF.Exp)
                        eng = nc.gpsimd if t % 4 == 2 else nc.vector
                        eng.tensor_tensor(
                            ex[:], ex[:],
                            mask_sb[:, t * 512:(t + 1) * 512], op=OP.mult)

                    def do_pv(t):
                        for par, k, src, off, n0, w in PV_BLOCKS[t]:
                            h = hs[par]
                            if k == 0:
                                pvs[par] = pvbc.tile([128, 512], f32, tag="pv",
                                                     name="pv")
                            nc.tensor.matmul(
                                pvs[par][:, n0:512],
                                v_ext[k][:, 128 * h:128 * h + 128],
                                exs[src][:, off:off + w],
                                start=(k == 0), stop=(k == (8 if par == 0 else 9)),
                                skip_group_check=True)

                    def tail(par):
                        # v2-proven chain: den row copy (64->64), PE broadcast
                        # to partitions 0-63, fast reciprocal, one DVE mult
                        g = gs[par]
                        pv = pvs[par]
                        den = prec.tile([65, 512], f32r, tag="den", name="den")
                        nc.vector.tensor_copy(den[64:65, :], pv[64:65, :])
                        bct = pvbc.tile([128, 512], f32, tag="pv", name="bct")
                        bc = bct[0:64, :]
                        nc.tensor.matmul(bc, ones64r[64:65, :], den[64:65, :],
                                         start=True, stop=True)
                        rec = prec.tile([64, 512], f32, tag="rec", name="rec")
                        nc.vector.reciprocal_approx_fast(rec[:], bc)
                        p0 = (g % 2) * 64
                        att = attT[g // 2][p0:p0 + 64, :]
                        nc.vector.tensor_tensor(att, pv[0:64, :], rec[:],
                                                op=OP.mult)

                    return do_sim, do_exp, do_pv, tail

                stream = []
                for m in range(8):
                    do_sim, do_exp, do_pv, tail = make_pair(m)
                    for t in range(NT):
                        post = []
                        if t == 8:
                            post.append(lambda tl=tail: tl(0))
                        if t == 16:
                            post.append(lambda tl=tail: tl(1))
                        stream.append((do_sim, do_exp, do_pv, t, post))
                for i, unit in enumerate(stream):
                    do_sim, do_exp, do_pv, t, post = unit
                    do_sim(t)
                    do_exp(t)
                    if i >= LAG:
                        p_sim, p_exp, p_pv, p_t, p_post = stream[i - LAG]
                        p_pv(p_t)
                        for fn in p_post:
                            fn()
                for i in range(len(stream) - LAG, len(stream)):
                    p_sim, p_exp, p_pv, p_t, p_post = stream[i]
                    p_pv(p_t)
                    for fn in p_post:
                        fn()

            # ---------------- phase 4: layernorm + out projection ----------------
            with (tc.tile_pool(name="lnt", bufs=1) as plnt,
                  tc.tile_pool(name="lnx", bufs=2) as plnx):
                mb16 = plnt.tile([128, NL], f16, tag="mb16")
                ib16 = plnt.tile([128, NL], f16, tag="ib16")
                with tc.tile_pool(name="psst", bufs=1, space="PSUM") as psst:
                    st_sum = psst.tile([1, NL], f32, tag="ssum")
                    st_sq = psst.tile([1, NL], f32, tag="ssq")
                    for e in range(EC):
                        sq = plnx.tile([128, NL], f16, tag="sqe")
                        nc.vector.tensor_tensor(sq[:], attT[e][:], attT[e][:],
                                                op=OP.mult)
                        nc.tensor.matmul(st_sum[:], ones_m1[:], attT[e][:],
                                         start=(e == 0), stop=(e == EC - 1))
                        nc.tensor.matmul(st_sq[:], ones_m1[:], sq[:],
                                         start=(e == 0), stop=(e == EC - 1))
                    mu = plnt.tile([1, NL], f32r, tag="mu")
                    nc.vector.tensor_scalar_mul(mu[:], st_sum[:], 1.0 / E)
                    var = plnt.tile([1, NL], f32, tag="var")
                    nc.vector.tensor_scalar_mul(var[:], st_sq[:], 1.0 / E)
                    mu2 = plnt.tile([1, NL], f32, tag="mu2")
                    nc.vector.tensor_tensor(mu2[:], mu[:], mu[:], op=OP.mult)
                    nc.vector.tensor_tensor(var[:], var[:], mu2[:], op=OP.subtract)
                    sd = plnt.tile([1, NL], f32, tag="sd")
                    nc.scalar.activation(sd[:], var[:], AF.Sqrt, bias=eps_c[:])
                    nc.vector.reciprocal_approx_fast(sd[:], sd[:])
                    isd = plnt.tile([1, NL], f32r, tag="isd")
                    nc.vector.tensor_copy(isd[:], sd[:])
                    mb = psst.tile([128, NL], f32, tag="mb")
                    nc.tensor.matmul(mb[:], ones_k1[:], mu[:], start=True, stop=True)
                    ib = psst.tile([128, NL], f32, tag="ib")
                    nc.tensor.matmul(ib[:], ones_k1[:], isd[:], start=True, stop=True)
                    nc.vector.tensor_copy(mb16[:], mb[:])
                    nc.vector.tensor_copy(ib16[:], ib[:])

                def apply_ln(e):
                    tmp = plnx.tile([128, NL], f16, tag="xn")
                    nc.vector.tensor_tensor(tmp[:], attT[e][:], mb16[:],
                                            op=OP.subtract)
                    nc.vector.tensor_tensor(tmp[:], tmp[:], ib16[:], op=OP.mult)
                    nc.vector.tensor_scalar(attT[e][:], tmp[:], lng_sb[e][:],
                                            lnb_sb[e][:], op0=OP.mult, op1=OP.add)

                for e in range(EC):
                    apply_ln(e)
                with (
                    tc.tile_pool(name="ysb", bufs=2) as pysb,
                    tc.tile_pool(name="psy", bufs=2, space="PSUM") as psy,
                ):
                    qi2 = 0
                    for nt in range(4):
                        pyA = psy.tile([128, 512], f32, tag="pyA", name="pyA")
                        pyB = psy.tile([128, 512], f32, tag="pyB", name="pyB")
                        pys2 = (pyA, pyB)
                        for e in range(EC):
                            for oc in range(2):
                                nc.tensor.matmul(
                                    pys2[oc][:],
                                    attT[e][:, nt * 128:(nt + 1) * 128],
                                    wo_sb[e][:, oc * 512:(oc + 1) * 512],
                                    start=(e == 0), stop=False)
                        for oc in range(2):
                            nc.tensor.matmul(pys2[oc][:], ones_k1h[:],
                                             bo_sb[0:1, oc * 512:(oc + 1) * 512],
                                             start=False, stop=True)
                            ys = pysb.tile([128, 512], f32, tag="ys")
                            if oc == 0:
                                nc.vector.tensor_copy(ys[:], pys2[oc][:])
                            else:
                                nc.scalar.activation(ys[:], pys2[oc][:],
                                                     AF.Identity)
                            hw_q(qi2).dma_start(
                                y[nt * 128:(nt + 1) * 128,
                                  oc * 512:(oc + 1) * 512], ys[:])
                            qi2 += 1
    nc.finalize()
    return nc


def _build_mask9(adjc):
    """adjc: [NL, N] int (causal&adj premasked). Returns the [128, PAIR_COLS]
    f16 mask in the pair layout."""
    at = adjc.T.astype(np.float16)  # [N, NL] = [s, n]
    mk = np.zeros((128, PAIR_COLS), np.float16)
    for (par, k), (t, off, n0, w) in PACK.items():
        mk[:, t * 512 + off:t * 512 + off + w] = at[k * 128:(k + 1) * 128, n0:512]
    return mk


def _tile_w(wT):
    """[E, M] f16 -> [128, EC*M] with [p, e*M+m] = wT[e*128+p, m]."""
    E_, M = wT.shape
    return np.ascontiguousarray(
        wT.reshape(EC, 128, M).transpose(1, 0, 2).reshape(128, EC * M))


def shard_inputs(inputs):
    q = np.asarray(inputs["query"], np.float32)
    k = np.asarray(inputs["key"], np.float32)
    v = np.asarray(inputs["value"], np.float32)
    adj = np.asarray(inputs["adj"], np.int32)
    WqT8 = (np.ascontiguousarray(np.asarray(inputs["Wq"], np.float32).T)
            / np.float32(8.0)).astype(np.float16)
    WkT = np.ascontiguousarray(np.asarray(inputs["Wk"], np.float32).T).astype(np.float16)
    WvT = np.ascontiguousarray(np.asarray(inputs["Wv"], np.float32).T).astype(np.float16)
    WoT = np.ascontiguousarray(np.asarray(inputs["Wo"], np.float32).T).astype(np.float16)
    bq8 = (np.asarray(inputs["bq"], np.float32) / np.float32(8.0)).reshape(EC, 128)
    bk2 = np.asarray(inputs["bk"], np.float32).reshape(2, 128)
    bv2 = np.asarray(inputs["bv"], np.float32).reshape(2, 128)
    bo1 = np.asarray(inputs["bo"], np.float32).reshape(1, E).astype(np.float16)
    lng = np.asarray(inputs["ln_g"], np.float32).reshape(EC, 128)
    lnb = np.asarray(inputs["ln_b"], np.float32).reshape(EC, 128)

    shared = dict(wq_d=_tile_w(WqT8), wk_d=_tile_w(WkT), wv_d=_tile_w(WvT),
                  wo_d=_tile_w(WoT), bq2d=bq8, bk2d=bk2,
                  bv2d=bv2, bo1=bo1, lng=lng, lnb=lnb,
                  ones1=np.ones((1, 128), np.float32),
                  ones1h=np.ones((1, 128), np.float16))
    per_b = []
    s_idx = np.arange(N)
    for b in range(B):
        kT = k[b].T.astype(np.float16).reshape(EC, 128, 4, 512)
        xk_d = np.ascontiguousarray(
            kT.transpose(1, 2, 0, 3).reshape(128, 4 * EC * 512))
        vT = v[b].T.astype(np.float16).reshape(EC, 128, 2, 1024)
        xv_d = np.ascontiguousarray(
            vT.transpose(1, 2, 0, 3).reshape(128, 2 * EC * 1024))
        per_b.append((xk_d, xv_d))
    in_maps = []
    for c in range(8):
        b, j = divmod(c, 4)
        rows = np.arange(j, N, 4)
        causal = s_idx[None, :] <= rows[:, None]          # [NL, N]
        adjc = np.where(causal, adj[b][rows], 0)
        m = dict(shared)
        m["xq_d"] = _tile_w(q[b][rows].T.astype(np.float16))
        m["xk_d"], m["xv_d"] = per_b[b]
        m["mask9"] = _build_mask9(adjc)
        in_maps.append(m)
    return in_maps


def _host_fixup(out, inputs):
    """Rows with no unmasked causal position get the reference's uniform-
    softmax-over-everything fallback, computed exactly on host."""
    adj = np.asarray(inputs["adj"])
    s_idx = np.arange(N)
    causal = s_idx[None, :] <= s_idx[:, None]
    for b in range(B):
        amr = np.where(((adj[b] != 0) & causal).sum(1) == 0)[0]
        if len(amr) == 0:
            continue
        v = np.asarray(inputs["value"][b], np.float64)
        Wv = np.asarray(inputs["Wv"], np.float64)
        bv = np.asarray(inputs["bv"], np.float64)
        vp = v @ Wv.T + bv                       # [N, KVE]
        mv = vp.mean(0)                          # [KVE]
        row = np.concatenate([mv[(k // G) * D:(k // G) * D + D] for k in range(HQ)])
        mu = row.mean()
        var = ((row - mu) ** 2).mean()
        rown = (row - mu) / np.sqrt(var + LN_EPS)
        rown = rown * np.asarray(inputs["ln_g"], np.float64) + np.asarray(
            inputs["ln_b"], np.float64)
        yrow = rown @ np.asarray(inputs["Wo"], np.float64).T + np.asarray(
            inputs["bo"], np.float64)
        out[b, amr, :] = yrow.astype(np.float32)
    return out


def unshard_outputs(results, inputs):
    out = np.empty((B, N, E), np.float32)
    for c in range(8):
        b, j = divmod(c, 4)
        out[b, j::4, :] = results[c]["y"]
    return _host_fixup(out, inputs)


def kernel(**inputs):
    from concourse.bass_utils import run_bass_kernel_spmd

    if "nc" not in _PROG_CACHE:
        _PROG_CACHE["nc"] = build_program()
    nc = _PROG_CACHE["nc"]
    in_maps = shard_inputs(inputs)
    res = run_bass_kernel_spmd(nc, in_maps, core_ids=list(range(8)))
    return unshard_outputs(res.results, inputs)


# revision 4
# speedup vs baseline: 1.1503x; 1.1443x over previous
"""GQA sparse-attention kernel for 8 Trainium2 NeuronCores — v3.

Sharding: data-parallel over batch (2) x sequence-parallel over query rows
(rows j::4 interleaved). No collectives.

v3 changes vs v2 (268us baseline):
  - perfect PSUM bank packing: the 16 causal s-blocks of each head pack as
    width-pairs summing to 512 (512 | 480+32 | ... | 288+224 | 256+256'),
    so a head PAIR fills exactly 17 one-bank [128,512] PSUM units with no
    padding.  One EXP and one mask-multiply per unit (8.5/head vs 10/head,
    all full-width).
  - deep software pipeline: psim pool of 5 units + shared pv/broadcast pool
    of 3 banks (8 total).  PV work is issued LAG=3 units behind sim, across
    head/pair boundaries, so PE semaphores are pre-satisfied: stalls reset
    the TRN2 PE DVFS ramp (measured: 512-col matmuls run 427ns at the 1.2GHz
    mid-pstate vs 216ns at 2.4GHz; ~4us of uninterrupted busy re-ramps).
  - tail per head: den-row copy (partition 64->64), PE broadcast matmul into
    the shared pv/bc pool, reciprocal_approx_fast, one DVE multiply.
  - host pre-tiles every weight/activation DRAM tensor into its SBUF layout
    so all input DMAs are contiguous >=4KB lines; ~13 big transfers round-
    robined over both HWDGE queues in consumption order (xk st-sliced so
    k-proj starts after ~1.5MB), wq/wv/consts on the SWDGE queue in parallel.
  - epilogue: stats chains + sqrt/recip LN chain, f16 LN apply, nt-outer
    out-projection so y stores stream to DRAM during compute.

Known behavior: run-to-run HW variance is large (~235-290us) — the PE DVFS
state interacts with cross-core power/HBM contention; medians ~250us.
"""

import os
import sys

import numpy as np

for _p in ("/opt/trn_rl_repo", "/root/.axon_site/_ro/trn_rl_repo"):
    if os.path.isdir(_p) and _p not in sys.path:
        sys.path.insert(0, _p)

B, N, E = 2, 2048, 1024
HQ, HK, D = 16, 4, 64
G = HQ // HK          # 4 query heads per kv head
KVE = HK * D          # 256
NL = N // 4           # 512 local query rows per core
SB = N // 128         # 16 s-blocks
EC = E // 128         # 8 embedding chunks
LN_EPS = 1e-5

# ---- head-pair PSUM packing ------------------------------------------------
# block k covers s in [128k, 128k+128); local query rows n >= 32k (global row
# 4n+j >= 128k), width w_k = 512-32k.  Emission orders pack banks exactly:
ORDER_EVEN = [0, 1, 15, 2, 14, 3, 13, 4, 12, 5, 11, 6, 10, 7, 9, 8]
ORDER_ODD = [8] + ORDER_EVEN[:-1]
NT = 17                                 # [128,512] PSUM units per head-pair
PAIR_COLS = NT * 512                    # 8704 layout cols, zero padding


def _build_pack():
    pack = {}
    pos = 0
    for par, order in ((0, ORDER_EVEN), (1, ORDER_ODD)):
        for k in order:
            w = 512 - 32 * k
            pack[(par, k)] = (pos // 512, pos % 512, 32 * k, w)
            pos += w
    assert pos == 8704
    return pack


PACK = _build_pack()
TW = [512] * NT
# unit -> [(par, k, off_in_unit, n0, w)] in layout order
TILE_BLOCKS = [[] for _ in range(NT)]
for (par, k), (t, off, n0, w) in sorted(PACK.items(), key=lambda i: (i[1][0], i[1][1])):
    TILE_BLOCKS[t].append((par, k, off, n0, w))
# PV emission per unit: entries are (par, k, src_unit, off, n0, w); odd
# head's k=8 PV is deferred to unit 9 so its k=0 start matmul (which zeroes
# the whole pv bank) always comes first
PV_BLOCKS = [[(par, k, t, off, n0, w) for par, k, off, n0, w in TILE_BLOCKS[t]]
             for t in range(NT)]
_odd8 = next(b for b in PV_BLOCKS[8] if b[0] == 1)
PV_BLOCKS[8].remove(_odd8)
PV_BLOCKS[9] = sorted(PV_BLOCKS[9] + [_odd8],
                      key=lambda b: (0 if b[1] == 0 else 1, b[3]))

_PROG_CACHE = {}


def build_program():
    import concourse.mybir as mybir
    import concourse.tile as tile
    from concourse import bacc

    dt = mybir.dt
    f32, f32r, f16, bf16 = dt.float32, dt.float32r, dt.float16, dt.bfloat16
    AF = mybir.ActivationFunctionType
    OP = mybir.AluOpType

    nc = bacc.Bacc("TRN2", target_bir_lowering=False, debug=False)

    def din(name, shape, dtp=f32):
        return nc.dram_tensor(name, shape, dtp, kind="ExternalInput").ap()

    # pre-tiled on host to the SBUF layout: [128, ...] with >=4KB lines
    xq_d = din("xq_d", [128, EC * NL], f16)
    xk_d = din("xk_d", [128, 4 * EC * 512], f16)   # st-major
    xv_d = din("xv_d", [128, 2 * EC * 1024], f16)  # half-major
    wq_d = din("wq_d", [128, EC * E], f16)         # pre-scaled by 1/8 on host
    wk_d = din("wk_d", [128, EC * KVE], f16)
    wv_d = din("wv_d", [128, EC * KVE], f16)
    wo_d = din("wo_d", [128, EC * E], f16)
    bq2d = din("bq2d", [EC, 128])          # bq/8
    bk2d = din("bk2d", [2, 128])
    bv2d = din("bv2d", [2, 128])
    bo1 = din("bo1", [1, E], f16)
    lng = din("lng", [EC, 128])
    lnb = din("lnb", [EC, 128])
    ones1 = din("ones1", [1, 128], f32r)
    ones1h = din("ones1h", [1, 128], f16)
    mask9 = din("mask9", [128, PAIR_COLS], f16)
    y = nc.dram_tensor("y", [NL, E], f32, kind="ExternalOutput").ap()

    # round-robin the two HWDGE queues in issue order
    def hw_q(i):
        return nc.sync if i % 2 == 0 else nc.scalar

    with tile.TileContext(nc) as tc, nc.allow_low_precision(
            "f16/f32r operands for PE fast-path matmuls are intentional"):
        with (
            tc.tile_pool(name="const", bufs=1) as pc,
            tc.tile_pool(name="persist", bufs=1) as pp,
        ):
            # ---- consolidated input DMAs, round-robin on the 2 HWDGE queues,
            # in consumption-priority order (each dma_start costs ~0.6us of
            # engine issue time, so few big transfers beat many small ones) ----
            qi = 0
            wk_all = pc.tile([128, EC * KVE], f16, tag="wk_all", name="wk_all")
            wk_sb = [wk_all[:, e * KVE:(e + 1) * KVE] for e in range(EC)]
            hw_q(qi).dma_start(wk_all[:], wk_d); qi += 1
            # xk in 4 st-slice tiles so k-proj st=0 starts after ~1.5MB
            xk_st = [pc.tile([128, EC * 512], f16, tag=f"xk_st{st}",
                             name=f"xk_st{st}") for st in range(4)]
            xk_sl = [[xk_st[st][:, e * 512:(e + 1) * 512]
                      for st in range(4)] for e in range(EC)]
            for st in range(4):
                hw_q(qi).dma_start(
                    xk_st[st][:], xk_d[:, st * EC * 512:(st + 1) * EC * 512])
                qi += 1
            wq_all = pc.tile([128, EC * E], f16, tag="wq_all", name="wq_all")
            wq_sb = [wq_all[:, e * E:(e + 1) * E] for e in range(EC)]
            nc.gpsimd.dma_start(wq_all[:], wq_d)
            xq_all = pc.tile([128, EC * NL], f16, tag="xq_all", name="xq_all")
            xq_sb = [xq_all[:, e * NL:(e + 1) * NL] for e in range(EC)]
            hw_q(qi).dma_start(xq_all[:], xq_d); qi += 1
            wv_all = pc.tile([128, EC * KVE], f16, tag="wv_all", name="wv_all")
            wv_sb = [wv_all[:, e * KVE:(e + 1) * KVE] for e in range(EC)]
            nc.gpsimd.dma_start(wv_all[:], wv_d)
            xv_st = [pc.tile([128, EC * 1024], f16, tag=f"xv_st{h}",
                             name=f"xv_st{h}") for h in range(2)]
            xv_sl = [[xv_st[st // 2][:, e * 1024 + (st % 2) * 512:
                                     e * 1024 + (st % 2) * 512 + 512]
                      for st in range(4)] for e in range(EC)]
            for half in range(2):
                hw_q(qi).dma_start(
                    xv_st[half][:],
                    xv_d[:, half * EC * 1024:(half + 1) * EC * 1024])
                qi += 1
            mask_sb = pp.tile([128, 8704], f16, tag="mask9", name="mask9")
            for half in range(2):
                hw_q(qi).dma_start(mask_sb[:, half * 4352:(half + 1) * 4352],
                                   mask9[:, half * 4352:(half + 1) * 4352])
                qi += 1
            wo_all = pp.tile([128, EC * E], f16, tag="wo_all", name="wo_all")
            wo_sb = [wo_all[:, e * E:(e + 1) * E] for e in range(EC)]
            hw_q(qi).dma_start(wo_all[:], wo_d); qi += 1

            # ---- tiny constants on the SWDGE queue ----
            bq_sb = [pc.tile([128, 1], f32, tag=f"bq{m}", name=f"bq{m}")
                     for m in range(EC)]
            for e in range(EC):
                nc.gpsimd.dma_start(bq_sb[e][:], bq2d[e:e + 1, :])
            bk_sb = [pc.tile([128, 1], f32, tag=f"bk{m}", name=f"bk{m}") for m in range(2)]
            bv_sb = [pc.tile([128, 1], f32, tag=f"bv{m}", name=f"bv{m}") for m in range(2)]
            for m in range(2):
                nc.gpsimd.dma_start(bk_sb[m][:], bk2d[m:m + 1, :])
                nc.gpsimd.dma_start(bv_sb[m][:], bv2d[m:m + 1, :])
            ones_k1 = pc.tile([1, 128], f32r, tag="ones_k1")
            nc.gpsimd.dma_start(ones_k1[:], ones1)
            ones64r = pc.tile([65, 64], f32r, tag="ones64r")
            nc.gpsimd.dma_start(ones64r[64:65, :], ones1[:, 0:64])
            ones_m1 = pc.tile([128, 1], f16, tag="ones_m1")
            nc.gpsimd.dma_start(ones_m1[:], ones1h)
            ones_k1h = pc.tile([1, 128], f16, tag="ones_k1h")
            nc.gpsimd.dma_start(ones_k1h[:], ones1h)
            lng_sb = [pp.tile([128, 1], f32, tag=f"lng{e}", name=f"lng{e}")
                      for e in range(EC)]
            lnb_sb = [pp.tile([128, 1], f32, tag=f"lnb{e}", name=f"lnb{e}")
                      for e in range(EC)]
            for e in range(EC):
                nc.gpsimd.dma_start(lng_sb[e][:], lng[e:e + 1, :])
                nc.gpsimd.dma_start(lnb_sb[e][:], lnb[e:e + 1, :])
            bo_sb = pp.tile([1, E], f16, tag="bo", name="bo")
            nc.gpsimd.dma_start(bo_sb[:], bo1)

            ident = pc.tile([128, 128], f16, tag="ident")
            from concourse.masks import make_identity
            make_identity(nc, ident[:])
            eps_c = pc.tile([1, 1], f32, tag="eps_c")
            nc.gpsimd.memset(eps_c[:], LN_EPS)

            # persistent activation tiles
            kT_sb = [pp.tile([128, N], f16, tag=f"kt{m}", name=f"kt{m}") for m in range(2)]
            v_ext = [pp.tile([128, 4 * 128], f16, tag=f"vx{k}", name=f"vx{k}")
                     for k in range(SB)]
            qp_sb = [pp.tile([128, NL], f16, tag=f"qp{m}", name=f"qp{m}")
                     for m in range(EC)]
            attT = [pp.tile([128, NL], f16, tag=f"at{e}", name=f"at{e}")
                    for e in range(EC)]
            _EVEN = [0, 1, 2, 3, 8, 9, 10, 11]    # heads whose kv head is even
            _ODD = [4, 5, 6, 7, 12, 13, 14, 15]

            def _qslot(g):
                if (g // G) % 2 == 0:
                    return _EVEN.index(g), 0
                return _ODD.index(g), 1

            # ---------------- phase 0: k projection ----------------
            with tc.tile_pool(name="psk2", bufs=2, space="PSUM") as psk2:
                for st in range(4):
                    sl = slice(st * 512, (st + 1) * 512)
                    for mt in range(2):
                        psk = psk2.tile([128, 512], f32, tag="psk")
                        for e in range(EC):
                            nc.tensor.matmul(
                                psk[:], wk_sb[e][:, mt * 128:(mt + 1) * 128],
                                xk_sl[e][st], start=(e == 0), stop=(e == EC - 1))
                        nc.scalar.activation(kT_sb[mt][:, sl], psk[:], AF.Identity,
                                             bias=bk_sb[mt][:], scale=1.0)

            # ---------------- phase 1: q projection ----------------
            with tc.tile_pool(name="psq", bufs=2, space="PSUM") as psq:
                for mt2 in range(EC // 2):
                    psA = psq.tile([128, NL], f32, tag="psqA", name="psqA")
                    psB = psq.tile([128, NL], f32, tag="psqB", name="psqB")
                    for e in range(EC):
                        for mt, ps in ((2 * mt2, psA), (2 * mt2 + 1, psB)):
                            nc.tensor.matmul(
                                ps[:], wq_sb[e][:, mt * 128:(mt + 1) * 128],
                                xq_sb[e], start=(e == 0), stop=(e == EC - 1))
                    for mt, ps in ((2 * mt2, psA), (2 * mt2 + 1, psB)):
                        for t in range(2):
                            g = 2 * mt + t
                            ti, slot = _qslot(g)
                            nc.scalar.activation(
                                qp_sb[ti][slot * 64:(slot + 1) * 64, :],
                                ps[t * 64:(t + 1) * 64, :], AF.Identity,
                                bias=bq_sb[mt][t * 64:(t + 1) * 64, :], scale=1.0)

            # ---------------- phase 2: v projection ----------------
            with (
                tc.tile_pool(name="vt", bufs=2) as pvt,
                tc.tile_pool(name="pskv", bufs=2, space="PSUM") as pskv,
                tc.tile_pool(name="pst", bufs=2, space="PSUM") as pst,
            ):
                for st in range(4):  # s-tiles of 512
                    for mt in range(2):
                        psv = pskv.tile([128, 512], f32, tag="psv")
                        for e in range(EC):
                            nc.tensor.matmul(
                                psv[:], wv_sb[e][:, mt * 128:(mt + 1) * 128],
                                xv_sl[e][st], start=(e == 0), stop=(e == EC - 1))
                        vt = pvt.tile([128, 512], f16, tag="vt")
                        nc.scalar.activation(vt[:], psv[:], AF.Identity,
                                             bias=bv_sb[mt][:], scale=1.0)
                        for ss in range(4):
                            k = st * 4 + ss
                            pt = pst.tile([128, 128], f16, tag="pt")
                            nc.tensor.transpose(pt[:], vt[:, ss * 128:(ss + 1) * 128],
                                                ident[:])
                            src = pt[:].rearrange("p (h x) -> p h x", h=2)
                            dst = v_ext[k][:].rearrange("p (h x) -> p h x", h=4)
                            nc.vector.tensor_copy(dst[:, 2 * mt:2 * mt + 2, 0:64], src)
                for k in range(SB):
                    pad = v_ext[k][:].rearrange("p (h x) -> p h x", h=4)[:, :, 64:128]
                    nc.gpsimd.memset(pad, 0.0)
                    one_col = v_ext[k][:].rearrange("p (h x) -> p h x", h=4)[:, :, 64:65]
                    nc.gpsimd.memset(one_col, 1.0)

            # ---------------- phase 3: attention ----------------
            # 512-col PSUM units, PV issued LAG units behind sim so every PE
            # semaphore is pre-satisfied and the DVFS busy-streak stays long
            LAG = 3
            with (
                tc.tile_pool(name="exs", bufs=LAG + 3) as pex,
                tc.tile_pool(name="recs", bufs=2) as prec,
                tc.tile_pool(name="psim", bufs=5, space="PSUM") as psim,
                tc.tile_pool(name="pvbc", bufs=3, space="PSUM") as pvbc,
            ):
                def make_pair(m):
                    """Returns per-unit work closures for head pair (2m, 2m+1)."""
                    gs = (2 * m, 2 * m + 1)
                    hs = tuple(g // G for g in gs)
                    kh = []
                    qg = []
                    for g in gs:
                        h = g // G
                        ti, slot = _qslot(g)
                        qg.append(qp_sb[ti][slot * 64:(slot + 1) * 64, :])
                        kh.append(kT_sb[h // 2][(h % 2) * 64:(h % 2) * 64 + 64, :])
                    pvs = [None, None]
                    sims = [None] * NT
                    exs = [None] * NT

                    def do_sim(t):
                        st_ = psim.tile([128, 512], f32, tag="sim", name="sim")
                        sims[t] = st_
                        for par, k, off, n0, w in TILE_BLOCKS[t]:
                            nc.tensor.matmul(
                                st_[:, off:off + w],
                                kh[par][:, k * 128:(k + 1) * 128],
                                qg[par][:, n0:512], start=True, stop=True)

                    def do_exp(t):
                        ex = pex.tile([128, 512], f16, tag="ex", name="ex")
                        exs[t] = ex
                        nc.scalar.activation(ex[:], sims[t][:], AF.Exp)
                        eng = nc.gpsimd if t % 4 == 2 else nc.vector
                        eng.tensor_tensor(
                            ex[:], ex[:],
                            mask_sb[:, t * 512:(t + 1) * 512], op=OP.mult)

                    def do_pv(t):
                        for par, k, src, off, n0, w in PV_BLOCKS[t]:
                            h = hs[par]
                            if k == 0:
                                pvs[par] = pvbc.tile([128, 512], f32, tag="pv",
                                                     name="pv")
                            nc.tensor.matmul(
                                pvs[par][:, n0:512],
                                v_ext[k][:, 128 * h:128 * h + 128],
                                exs[src][:, off:off + w],
                                start=(k == 0), stop=(k == (8 if par == 0 else 9)),
                                skip_group_check=True)

                    def tail(par):
                        # v2-proven chain: den row copy (64->64), PE broadcast
                        # to partitions 0-63, fast reciprocal, one DVE mult
                        g = gs[par]
                        pv = pvs[par]
                        den = prec.tile([65, 512], f32r, tag="den", name="den")
                        nc.vector.tensor_copy(den[64:65, :], pv[64:65, :])
                        bct = pvbc.tile([128, 512], f32, tag="pv", name="bct")
                        bc = bct[0:64, :]
                        nc.tensor.matmul(bc, ones64r[64:65, :], den[64:65, :],
                                         start=True, stop=True)
                        rec = prec.tile([64, 512], f32, tag="rec", name="rec")
                        nc.vector.reciprocal_approx_fast(rec[:], bc)
                        p0 = (g % 2) * 64
                        att = attT[g // 2][p0:p0 + 64, :]
                        nc.vector.tensor_tensor(att, pv[0:64, :], rec[:],
                                                op=OP.mult)

                    return do_sim, do_exp, do_pv, tail

                stream = []
                for m in range(8):
                    do_sim, do_exp, do_pv, tail = make_pair(m)
                    for t in range(NT):
                        post = []
                        if t == 8:
                            post.append(lambda tl=tail: tl(0))
                        if t == 16:
                            post.append(lambda tl=tail: tl(1))
                        stream.append((do_sim, do_exp, do_pv, t, post))
                for i, unit in enumerate(stream):
                    do_sim, do_exp, do_pv, t, post = unit
                    do_sim(t)
                    do_exp(t)
                    if i >= LAG:
                        p_sim, p_exp, p_pv, p_t, p_post = stream[i - LAG]
                        p_pv(p_t)
                        for fn in p_post:
                            fn()
                for i in range(len(stream) - LAG, len(stream)):
                    p_sim, p_exp, p_pv, p_t, p_post = stream[i]
                    p_pv(p_t)
                    for fn in p_post:
                        fn()

            # ---------------- phase 4: layernorm + out projection ----------------
            with (tc.tile_pool(name="lnt", bufs=1) as plnt,
                  tc.tile_pool(name="lnx", bufs=2) as plnx):
                mb16 = plnt.tile([128, NL], f16, tag="mb16")
                ib16 = plnt.tile([128, NL], f16, tag="ib16")
                with tc.tile_pool(name="psst", bufs=1, space="PSUM") as psst:
                    st_sum = psst.tile([1, NL], f32, tag="ssum")
                    st_sq = psst.tile([1, NL], f32, tag="ssq")
                    for e in range(EC):
                        sq = plnx.tile([128, NL], f16, tag="sqe")
                        nc.vector.tensor_tensor(sq[:], attT[e][:], attT[e][:],
                                                op=OP.mult)
                        nc.tensor.matmul(st_sum[:], ones_m1[:], attT[e][:],
                                         start=(e == 0), stop=(e == EC - 1))
                        nc.tensor.matmul(st_sq[:], ones_m1[:], sq[:],
                                         start=(e == 0), stop=(e == EC - 1))
                    mu = plnt.tile([1, NL], f32r, tag="mu")
                    nc.vector.tensor_scalar_mul(mu[:], st_sum[:], 1.0 / E)
                    var = plnt.tile([1, NL], f32, tag="var")
                    nc.vector.tensor_scalar_mul(var[:], st_sq[:], 1.0 / E)
                    mu2 = plnt.tile([1, NL], f32, tag="mu2")
                    nc.vector.tensor_tensor(mu2[:], mu[:], mu[:], op=OP.mult)
                    nc.vector.tensor_tensor(var[:], var[:], mu2[:], op=OP.subtract)
                    sd = plnt.tile([1, NL], f32, tag="sd")
                    nc.scalar.activation(sd[:], var[:], AF.Sqrt, bias=eps_c[:])
                    nc.vector.reciprocal_approx_fast(sd[:], sd[:])
                    isd = plnt.tile([1, NL], f32r, tag="isd")
                    nc.vector.tensor_copy(isd[:], sd[:])
                    mb = psst.tile([128, NL], f32, tag="mb")
                    nc.tensor.matmul(mb[:], ones_k1[:], mu[:], start=True, stop=True)
                    ib = psst.tile([128, NL], f32, tag="ib")
                    nc.tensor.matmul(ib[:], ones_k1[:], isd[:], start=True, stop=True)
                    nc.vector.tensor_copy(mb16[:], mb[:])
                    nc.vector.tensor_copy(ib16[:], ib[:])

                def apply_ln(e):
                    tmp = plnx.tile([128, NL], f16, tag="xn")
                    nc.vector.tensor_tensor(tmp[:], attT[e][:], mb16[:],
                                            op=OP.subtract)
                    nc.vector.tensor_tensor(tmp[:], tmp[:], ib16[:], op=OP.mult)
                    nc.vector.tensor_scalar(attT[e][:], tmp[:], lng_sb[e][:],
                                            lnb_sb[e][:], op0=OP.mult, op1=OP.add)

                for e in range(EC):
                    apply_ln(e)
                with (
                    tc.tile_pool(name="ysb", bufs=2) as pysb,
                    tc.tile_pool(name="psy", bufs=2, space="PSUM") as psy,
                ):
                    qi2 = 0
                    for nt in range(4):
                        pyA = psy.tile([128, 512], f32, tag="pyA", name="pyA")
                        pyB = psy.tile([128, 512], f32, tag="pyB", name="pyB")
                        pys2 = (pyA, pyB)
                        for e in range(EC):
                            for oc in range(2):
                                nc.tensor.matmul(
                                    pys2[oc][:],
                                    attT[e][:, nt * 128:(nt + 1) * 128],
                                    wo_sb[e][:, oc * 512:(oc + 1) * 512],
                                    start=(e == 0), stop=False)
                        for oc in range(2):
                            nc.tensor.matmul(pys2[oc][:], ones_k1h[:],
                                             bo_sb[0:1, oc * 512:(oc + 1) * 512],
                                             start=False, stop=True)
                            ys = pysb.tile([128, 512], f32, tag="ys")
                            if oc == 0:
                                nc.vector.tensor_copy(ys[:], pys2[oc][:])
                            else:
                                nc.scalar.activation(ys[:], pys2[oc][:],
                                                     AF.Identity)
                            hw_q(qi2).dma_start(
                                y[nt * 128:(nt + 1) * 128,
                                  oc * 512:(oc + 1) * 512], ys[:])
                            qi2 += 1
    nc.finalize()
    return nc


def _build_mask9(adjc):
    """adjc: [NL, N] int (causal&adj premasked). Returns the [128, PAIR_COLS]
    f16 mask in the pair layout."""
    at = adjc.T.astype(np.float16)  # [N, NL] = [s, n]
    mk = np.zeros((128, PAIR_COLS), np.float16)
    for (par, k), (t, off, n0, w) in PACK.items():
        mk[:, t * 512 + off:t * 512 + off + w] = at[k * 128:(k + 1) * 128, n0:512]
    return mk


def _tile_w(wT):
    """[E, M] f16 -> [128, EC*M] with [p, e*M+m] = wT[e*128+p, m]."""
    E_, M = wT.shape
    return np.ascontiguousarray(
        wT.reshape(EC, 128, M).transpose(1, 0, 2).reshape(128, EC * M))


def shard_inputs(inputs):
    q = np.asarray(inputs["query"], np.float32)
    k = np.asarray(inputs["key"], np.float32)
    v = np.asarray(inputs["value"], np.float32)
    adj = np.asarray(inputs["adj"], np.int32)
    WqT8 = (np.ascontiguousarray(np.asarray(inputs["Wq"], np.float32).T)
            / np.float32(8.0)).astype(np.float16)
    WkT = np.ascontiguousarray(np.asarray(inputs["Wk"], np.float32).T).astype(np.float16)
    WvT = np.ascontiguousarray(np.asarray(inputs["Wv"], np.float32).T).astype(np.float16)
    WoT = np.ascontiguousarray(np.asarray(inputs["Wo"], np.float32).T).astype(np.float16)
    bq8 = (np.asarray(inputs["bq"], np.float32) / np.float32(8.0)).reshape(EC, 128)
    bk2 = np.asarray(inputs["bk"], np.float32).reshape(2, 128)
    bv2 = np.asarray(inputs["bv"], np.float32).reshape(2, 128)
    bo1 = np.asarray(inputs["bo"], np.float32).reshape(1, E).astype(np.float16)
    lng = np.asarray(inputs["ln_g"], np.float32).reshape(EC, 128)
    lnb = np.asarray(inputs["ln_b"], np.float32).reshape(EC, 128)

    shared = dict(wq_d=_tile_w(WqT8), wk_d=_tile_w(WkT), wv_d=_tile_w(WvT),
                  wo_d=_tile_w(WoT), bq2d=bq8, bk2d=bk2,
                  bv2d=bv2, bo1=bo1, lng=lng, lnb=lnb,
                  ones1=np.ones((1, 128), np.float32),
                  ones1h=np.ones((1, 128), np.float16))
    per_b = []
    s_idx = np.arange(N)
    for b in range(B):
        kT = k[b].T.astype(np.float16).reshape(EC, 128, 4, 512)
        xk_d = np.ascontiguousarray(
            kT.transpose(1, 2, 0, 3).reshape(128, 4 * EC * 512))
        vT = v[b].T.astype(np.float16).reshape(EC, 128, 2, 1024)
        xv_d = np.ascontiguousarray(
            vT.transpose(1, 2, 0, 3).reshape(128, 2 * EC * 1024))
        per_b.append((xk_d, xv_d))
    in_maps = []
    for c in range(8):
        b, j = divmod(c, 4)
        rows = np.arange(j, N, 4)
        causal = s_idx[None, :] <= rows[:, None]          # [NL, N]
        adjc = np.where(causal, adj[b][rows], 0)
        m = dict(shared)
        m["xq_d"] = _tile_w(q[b][rows].T.astype(np.float16))
        m["xk_d"], m["xv_d"] = per_b[b]
        m["mask9"] = _build_mask9(adjc)
        in_maps.append(m)
    return in_maps


def _host_fixup(out, inputs):
    """Rows with no unmasked causal position get the reference's uniform-
    softmax-over-everything fallback, computed exactly on host."""
    adj = np.asarray(inputs["adj"])
    s_idx = np.arange(N)
    causal = s_idx[None, :] <= s_idx[:, None]
    for b in range(B):
        amr = np.where(((adj[b] != 0) & causal).sum(1) == 0)[0]
        if len(amr) == 0:
            continue
        v = np.asarray(inputs["value"][b], np.float64)
        Wv = np.asarray(inputs["Wv"], np.float64)
        bv = np.asarray(inputs["bv"], np.float64)
        vp = v @ Wv.T + bv                       # [N, KVE]
        mv = vp.mean(0)                          # [KVE]
        row = np.concatenate([mv[(k // G) * D:(k // G) * D + D] for k in range(HQ)])
        mu = row.mean()
        var = ((row - mu) ** 2).mean()
        rown = (row - mu) / np.sqrt(var + LN_EPS)
        rown = rown * np.asarray(inputs["ln_g"], np.float64) + np.asarray(
            inputs["ln_b"], np.float64)
        yrow = rown @ np.asarray(inputs["Wo"], np.float64).T + np.asarray(
            inputs["bo"], np.float64)
        out[b, amr, :] = yrow.astype(np.float32)
    return out


def unshard_outputs(results, inputs):
    out = np.empty((B, N, E), np.float32)
    for c in range(8):
        b, j = divmod(c, 4)
        out[b, j::4, :] = results[c]["y"]
    return _host_fixup(out, inputs)


def kernel(**inputs):
    from concourse.bass_utils import run_bass_kernel_spmd

    if "nc" not in _PROG_CACHE:
        _PROG_CACHE["nc"] = build_program()
    nc = _PROG_CACHE["nc"]
    in_maps = shard_inputs(inputs)
    res = run_bass_kernel_spmd(nc, in_maps, core_ids=list(range(8)))
    return unshard_outputs(res.results, inputs)
